# revision 1
# baseline (speedup 1.0000x reference)
"""TRN2 Bass kernel for nn_CoreAttention_34875134444341.

Strategy (8 NeuronCores, no collectives):
  - Data-parallel over batch (4) x causal-balanced query-row split (2).
  - Each core: Q projection for its 1024 query rows (zig-zag tile
    assignment balances causal attention work) spilled to DRAM scratch;
    full K/V projections for its batch kept resident in SBUF; block
    attention in "transposed" orientation (keys on the partition axis)
    so no on-chip transposes are needed; final Wo matmul row-parallel
    (no reduction across cores).
  - All matmuls run as float32r (full-rate fp32 on the PE array) except
    the Wo stage which runs bf16.
  - Host gathers per-core outputs and inverse-permutes rows.
"""

import sys

sys.path.insert(0, "/opt/trn_rl_repo")

import numpy as np
import ml_dtypes

B, S, D = 4, 2048, 2048
H, HKV, DK = 16, 4, 128
RQ = RKV = 512
GROUP = H // HKV
P = 128

TILE_R = 256  # query rows per slot
KB = 128  # keys per block
NB_SCHED = [16, 12, 8, 4]  # key blocks per slot (same on every core)
TILE_ASSIGN = {0: [7, 5, 2, 0], 1: [6, 4, 3, 1]}  # slot -> query tile

ROWS_PER_CORE = 4 * TILE_R  # 1024

_CACHE = {}
TRACE = False
LAST_RESULT = None


def _rows_sched(parity):
    return np.concatenate(
        [np.arange(t * TILE_R, (t + 1) * TILE_R) for t in TILE_ASSIGN[parity]]
    )


def _make_mask(parity):
    """[128 key_in_block, 4 slots, 4 blocks, 512 (same mask for 2 heads)]."""
    m = np.zeros((KB, 4, 4, TILE_R), np.float32)
    for s in range(4):
        t = TILE_ASSIGN[parity][s]
        nb = NB_SCHED[s]
        row_g = t * TILE_R + np.arange(TILE_R)
        for j in range(4):  # last four blocks of the slot's schedule
            blk = nb - 4 + j
            key_g = blk * KB + np.arange(KB)
            bad = key_g[:, None] > row_g[None, :]
            m[:, s, j][bad] = -1e30
    return np.concatenate([m, m], axis=-1)


def _build_nc():
    import concourse.tile as tile
    from concourse import bacc, mybir

    f32 = mybir.dt.float32
    f32r = mybir.dt.float32r
    bf16 = mybir.dt.bfloat16
    Exp = mybir.ActivationFunctionType.Exp
    Mult = mybir.AluOpType.mult
    Add = mybir.AluOpType.add

    nc = bacc.Bacc("TRN2", target_bir_lowering=False, debug=False)

    xTkv = nc.dram_tensor("xTkv", [D, S], f32r, kind="ExternalInput")
    xTq = nc.dram_tensor("xTq", [D, ROWS_PER_CORE], f32r, kind="ExternalInput")
    wq1 = nc.dram_tensor("wq1", [D, RQ], f32r, kind="ExternalInput")
    wq2 = nc.dram_tensor("wq2", [RQ, H * DK], f32r, kind="ExternalInput")
    wk1 = nc.dram_tensor("wk1", [D, RKV], f32r, kind="ExternalInput")
    wk2 = nc.dram_tensor("wk2", [RKV, HKV * DK], f32r, kind="ExternalInput")
    wv1 = nc.dram_tensor("wv1", [D, RKV], f32r, kind="ExternalInput")
    wv2 = nc.dram_tensor("wv2", [RKV, HKV * DK], f32r, kind="ExternalInput")
    wo = nc.dram_tensor("wo", [D, D], bf16, kind="ExternalInput")
    maskin = nc.dram_tensor("maskin", [KB, 4, 4, 2 * TILE_R], f32, kind="ExternalInput")
    ones_in = nc.dram_tensor("ones_in", [P, 1], f32r, kind="ExternalInput")
    out = nc.dram_tensor("out", [ROWS_PER_CORE, D], f32, kind="ExternalOutput")

    qT_dram = nc.dram_tensor("qT_scratch", [H, P, ROWS_PER_CORE], f32r)

    xTkv_t = xTkv.rearrange("(dc p) s -> p dc s", p=P)  # [128, 16, 2048]
    xTq_t = xTq.rearrange("(dc p) r -> p dc r", p=P)  # [128, 16, 1024]
    wq1_t = wq1.rearrange("(dc p) r -> p dc r", p=P)  # [128, 16, 512]
    wk1_t = wk1.rearrange("(dc p) r -> p dc r", p=P)
    wv1_t = wv1.rearrange("(dc p) r -> p dc r", p=P)
    wq2_t = wq2.rearrange("(rc p) h -> p rc h", p=P)  # [128, 4, 2048]
    wk2_t = wk2.rearrange("(rc p) h -> p rc h", p=P)  # [128, 4, 512]
    wv2_t = wv2.rearrange("(rc p) h -> p rc h", p=P)
    wo_t = wo.rearrange("(hc p) o -> p hc o", p=P)  # [128, 16, 2048]

    with tile.TileContext(nc) as tc:
        with tc.tile_pool(name="persist", bufs=1) as persist:
            ones_sb = persist.tile([P, 1], f32r)
            nc.sync.dma_start(ones_sb[:], ones_in[:])

            # ------- Phase 1: Q projection -> DRAM scratch ----------------
            with (
                tc.tile_pool(name="q_w", bufs=1) as q_w,
                tc.tile_pool(name="q_x", bufs=2) as q_x,
                tc.tile_pool(name="q_mid", bufs=1) as q_mid,
                tc.tile_pool(name="q_out", bufs=3) as q_out,
                tc.tile_pool(name="q_ps", bufs=4, space="PSUM") as q_ps,
            ):
                wq1_sb = q_w.tile([P, 16, RQ], f32r)
                nc.sync.dma_start(wq1_sb[:], wq1_t)
                xts = []
                for tcn in range(2):
                    xt = q_x.tile([P, 16, 512], f32r, tag="xtq")
                    nc.sync.dma_start(xt[:], xTq_t[:, :, tcn * 512 : (tcn + 1) * 512])
                    xts.append(xt)
                wq2_sb = q_w.tile([P, 4, H * DK], f32r)
                nc.sync.dma_start(wq2_sb[:], wq2_t)

                q1t = q_mid.tile([P, 4, ROWS_PER_CORE], f32r)
                for tcn in range(2):
                    xt = xts[tcn]
                    for rc in range(4):
                        ps_q = q_ps.tile([P, 512], f32, tag="psq1")
                        for dc in range(16):
                            nc.tensor.matmul(
                                ps_q[:],
                                wq1_sb[:, dc, rc * P : (rc + 1) * P],
                                xt[:, dc],
                                start=(dc == 0),
                                stop=(dc == 15),
                            )
                        nc.any.tensor_copy(
                            q1t[:, rc, tcn * 512 : (tcn + 1) * 512], ps_q[:]
                        )
                for h in range(H):
                    for tcn in range(2):
                        ps_qT = q_ps.tile([P, 512], f32, tag="psq2")
                        for rc in range(4):
                            nc.tensor.matmul(
                                ps_qT[:],
                                wq2_sb[:, rc, h * P : (h + 1) * P],
                                q1t[:, rc, tcn * 512 : (tcn + 1) * 512],
                                start=(rc == 0),
                                stop=(rc == 3),
                            )
                        qbounce = q_out.tile([P, 512], f32r, tag="qb")
                        nc.any.tensor_copy(qbounce[:], ps_qT[:])
                        nc.sync.dma_start(
                            qT_dram[h, :, tcn * 512 : (tcn + 1) * 512], qbounce[:]
                        )

            # ------- kT/v stay resident from here on ----------------------
            with tc.tile_pool(name="kv_keep", bufs=1) as kv_keep:
                kT_sb = kv_keep.tile([P, HKV, S], f32r)
                v_sb = kv_keep.tile([P, S // P, HKV * DK], f32r)

                # ---- Phase 2: K then V projections (resident outputs) ----
                with (
                    tc.tile_pool(name="kv_w1", bufs=1) as kv_w1,
                    tc.tile_pool(name="kv_w2", bufs=1) as kv_w2,
                    tc.tile_pool(name="kv_x", bufs=2) as kv_x,
                    tc.tile_pool(name="kv_mid", bufs=1) as kv_mid,
                    tc.tile_pool(name="kv_ps", bufs=4, space="PSUM") as kv_ps,
                ):
                    for which in range(2):  # 0 = K, 1 = V
                        w1_t, w2_t = (wk1_t, wk2_t) if which == 0 else (wv1_t, wv2_t)
                        w1_sb = kv_w1.tile([P, 16, RKV], f32r, tag="w1")
                        for dq in range(4):
                            nc.sync.dma_start(
                                w1_sb[:, dq * 4 : (dq + 1) * 4], w1_t[:, dq * 4 : (dq + 1) * 4]
                            )
                        w2_sb = kv_w2.tile([P, 4, HKV * DK], f32r, tag="w2")
                        nc.sync.dma_start(w2_sb[:], w2_t)

                        for tcn in range(4):  # token 512-chunks
                            xt = kv_x.tile([P, 16, 512], f32r, tag="xt")
                            nc.sync.dma_start(
                                xt[:], xTkv_t[:, :, tcn * 512 : (tcn + 1) * 512]
                            )
                            mid = kv_mid.tile([P, 4, 512], f32r, tag="mid")
                            for rc in range(4):
                                ps_1 = kv_ps.tile([P, 512], f32, tag="ps1")
                                for dc in range(16):
                                    nc.tensor.matmul(
                                        ps_1[:],
                                        w1_sb[:, dc, rc * P : (rc + 1) * P],
                                        xt[:, dc],
                                        start=(dc == 0),
                                        stop=(dc == 15),
                                    )
                                nc.any.tensor_copy(mid[:, rc], ps_1[:])

                            if which == 0:
                                for hc in range(HKV):
                                    ps_2 = kv_ps.tile([P, 512], f32, tag="ps2")
                                    for rc in range(4):
                                        nc.tensor.matmul(
                                            ps_2[:],
                                            w2_sb[:, rc, hc * P : (hc + 1) * P],
                                            mid[:, rc],
                                            start=(rc == 0),
                                            stop=(rc == 3),
                                        )
                                    nc.any.tensor_copy(
                                        kT_sb[:, hc, tcn * 512 : (tcn + 1) * 512],
                                        ps_2[:],
                                    )
                            else:
                                for i in range(4):
                                    ps_2 = kv_ps.tile([P, 512], f32, tag="ps2")
                                    for rc in range(4):
                                        nc.tensor.matmul(
                                            ps_2[:],
                                            mid[:, rc, i * P : (i + 1) * P],
                                            w2_sb[:, rc],
                                            start=(rc == 0),
                                            stop=(rc == 3),
                                        )
                                    nc.any.tensor_copy(v_sb[:, tcn * 4 + i], ps_2[:])

                # ---- Phases 3+4 share attn_all ----
                with tc.tile_pool(name="attn_keep", bufs=1) as attn_keep:
                    attn_all = attn_keep.tile([P, 4, H, TILE_R], bf16)

                    with (
                        tc.tile_pool(name="at_m", bufs=1) as at_m,
                        tc.tile_pool(name="at_q", bufs=2) as at_q,
                        tc.tile_pool(name="at_e", bufs=4) as at_e,
                        tc.tile_pool(name="at_small", bufs=4) as at_small,
                        tc.tile_pool(name="at_ps", bufs=4, space="PSUM") as at_ps,
                        tc.tile_pool(name="at_ps_acc", bufs=2, space="PSUM") as at_ps_acc,
                        tc.tile_pool(name="at_ps_sum", bufs=2, space="PSUM") as at_ps_sum,
                    ):
                        mask_sb = at_m.tile([P, 4, 4, 2 * TILE_R], f32)
                        nc.sync.dma_start(mask_sb[:], maskin[:])

                        for s in (3, 2, 1, 0):
                            nb = NB_SCHED[s]
                            qT_sl = at_q.tile([P, H, TILE_R], f32r, tag="qsl")
                            nc.sync.dma_start(
                                qT_sl[:],
                                qT_dram.rearrange("h p r -> p h r")[
                                    :, :, s * TILE_R : (s + 1) * TILE_R
                                ],
                            )
                            for hp in range(H // 2):  # head pairs share kvh
                                h0 = 2 * hp
                                kvh = h0 // GROUP
                                # packed accumulator for both heads; only the
                                # first MM carries start=True (bank-wide clear)
                                ps_at = at_ps_acc.tile([P, 2 * TILE_R], f32, tag="at")
                                ps_sum = at_ps_sum.tile(
                                    [1, 2 * TILE_R], f32, tag="sum"
                                )
                                for b in range(nb):
                                    # one kT-block LDWEIGHTS feeds both heads
                                    ps_sc = at_ps.tile([P, 2 * TILE_R], f32, tag="sc")
                                    for i in range(2):
                                        nc.tensor.matmul(
                                            ps_sc[:, i * TILE_R : (i + 1) * TILE_R],
                                            kT_sb[:, kvh, b * KB : (b + 1) * KB],
                                            qT_sl[:, h0 + i],
                                            start=True,
                                            stop=True,
                                        )
                                    j = b - (nb - 4)
                                    if j >= 0:
                                        nc.vector.tensor_tensor(
                                            ps_sc[:], ps_sc[:], mask_sb[:, s, j], Add
                                        )
                                    e_sb = at_e.tile([P, 2 * TILE_R], f32r, tag="e")
                                    nc.scalar.activation(e_sb[:], ps_sc[:], Exp)
                                    for i in range(2):
                                        nc.tensor.matmul(
                                            ps_at[:, i * TILE_R : (i + 1) * TILE_R],
                                            v_sb[:, b, kvh * DK : (kvh + 1) * DK],
                                            e_sb[:, i * TILE_R : (i + 1) * TILE_R],
                                            start=(b == 0 and i == 0),
                                            stop=(b == nb - 1),
                                        )
                                    nc.tensor.matmul(
                                        ps_sum[:],
                                        ones_sb[:],
                                        e_sb[:],
                                        start=(b == 0),
                                        stop=(b == nb - 1),
                                    )
                                rec_sb = at_small.tile([1, 2 * TILE_R], f32, tag="rec")
                                nc.vector.reciprocal(rec_sb[:], ps_sum[:])
                                bc_sb = at_small.tile([P, 2 * TILE_R], f32, tag="bc")
                                nc.gpsimd.partition_broadcast(bc_sb[:], rec_sb[:])
                                for i in range(2):
                                    nc.vector.tensor_tensor(
                                        attn_all[:, s, h0 + i],
                                        ps_at[:, i * TILE_R : (i + 1) * TILE_R],
                                        bc_sb[:, i * TILE_R : (i + 1) * TILE_R],
                                        Mult,
                                    )

                    # ---- Phase 4: Wo ----
                    with (
                        tc.tile_pool(name="wo_w", bufs=2) as wo_w,
                        tc.tile_pool(name="wo_out", bufs=3) as wo_out,
                        tc.tile_pool(name="wo_ps", bufs=3, space="PSUM") as wo_ps,
                    ):
                        for oc in range(4):
                            wo_sb = wo_w.tile([P, 16, 512], bf16, tag="woc")
                            nc.sync.dma_start(
                                wo_sb[:], wo_t[:, :, oc * 512 : (oc + 1) * 512]
                            )
                            for rc in range(8):
                                s, half = rc // 2, rc % 2
                                ps_o = wo_ps.tile([P, 512], f32, tag="o")
                                for hc in range(16):
                                    nc.tensor.matmul(
                                        ps_o[:],
                                        attn_all[
                                            :, s, hc, half * P : (half + 1) * P
                                        ],
                                        wo_sb[:, hc],
                                        start=(hc == 0),
                                        stop=(hc == 15),
                                    )
                                o_sb = wo_out.tile([P, 512], f32, tag="osb")
                                nc.vector.tensor_copy(o_sb[:], ps_o[:])
                                nc.sync.dma_start(
                                    out[
                                        rc * P : (rc + 1) * P,
                                        oc * 512 : (oc + 1) * 512,
                                    ],
                                    o_sb[:],
                                )

    nc.finalize()
    return nc


def kernel(x, Wq1, Wq2, Wk1, Wk2, Wv1, Wv2, Wo):
    global LAST_RESULT
    from concourse.bass_utils import run_bass_kernel_spmd

    x = np.asarray(x, dtype=np.float32)
    Wq1 = np.asarray(Wq1, dtype=np.float32)
    Wq2 = np.asarray(Wq2, dtype=np.float32)
    Wk1 = np.asarray(Wk1, dtype=np.float32)
    Wk2 = np.asarray(Wk2, dtype=np.float32)
    Wv1 = np.asarray(Wv1, dtype=np.float32)
    Wv2 = np.asarray(Wv2, dtype=np.float32)
    Wo = np.asarray(Wo, dtype=np.float32)

    if "nc" not in _CACHE:
        _CACHE["nc"] = _build_nc()
    nc = _CACHE["nc"]

    wq2s = (Wq2 / np.sqrt(DK)).astype(np.float32)
    wo_bf = Wo.astype(ml_dtypes.bfloat16)
    masks = {p: _make_mask(p) for p in range(2)}
    rows = {p: _rows_sched(p) for p in range(2)}
    ones_np = np.ones((P, 1), np.float32)

    in_maps = []
    for core in range(8):
        batch, parity = core // 2, core % 2
        xb = x[batch]
        in_maps.append(
            {
                "xTkv": np.ascontiguousarray(xb.T),
                "xTq": np.ascontiguousarray(xb[rows[parity]].T),
                "wq1": Wq1,
                "wq2": wq2s,
                "wk1": Wk1,
                "wk2": Wk2,
                "wv1": Wv1,
                "wv2": Wv2,
                "wo": wo_bf,
                "maskin": masks[parity],
                "ones_in": ones_np,
            }
        )

    res = run_bass_kernel_spmd(nc, in_maps, core_ids=list(range(8)), trace=TRACE)
    LAST_RESULT = res

    out_full = np.empty((B, S, D), np.float32)
    for core in range(8):
        batch, parity = core // 2, core % 2
        out_full[batch][rows[parity]] = res.results[core]["out"]
    return out_full



# revision 2
# speedup vs baseline: 1.4559x; 1.4559x over previous
"""TRN2 Bass kernel for nn_CoreAttention_34875134444341 (v2).

Strategy (8 NeuronCores, no collectives):
  - Data-parallel over batch (4) x causal-balanced query-row split (2).
  - All matmul operands bf16 (FWL weight loads, half DMA/SBUF); PSUM
    accumulation stays fp32.
  - qT kept resident in SBUF (no DRAM spill/reload).
  - Attention: head pairs share one N=512 matmul; key blocks processed
    in pairs so exp runs on [128,1024]; softmax denominators via a
    ones[128,128] stationary matmul that lands broadcast across all
    PSUM partitions (no 1-lane reciprocal, no gpsimd broadcast);
    reciprocal_approx_fast + one multiply finish the normalization.
  - Weight/x DMAs split into pieces and prefetched across phase
    boundaries so the PE never waits long.
"""

import sys

sys.path.insert(0, "/opt/trn_rl_repo")

import numpy as np
import ml_dtypes

B, S, D = 4, 2048, 2048
H, HKV, DK = 16, 4, 128
RQ = RKV = 512
GROUP = H // HKV
P = 128

TILE_R = 256  # query rows per slot
KB = 128  # keys per block
NB_SCHED = [16, 12, 8, 4]  # union key blocks per slot (both parities)
TILE_ASSIGN = {0: [7, 5, 2, 0], 1: [6, 4, 3, 1]}  # slot -> query tile

ROWS_PER_CORE = 4 * TILE_R  # 1024

_CACHE = {}
TRACE = False
LAST_RESULT = None

BF16 = ml_dtypes.bfloat16


def _rows_sched(parity):
    return np.concatenate(
        [np.arange(t * TILE_R, (t + 1) * TILE_R) for t in TILE_ASSIGN[parity]]
    )


def _make_mask(parity):
    """[128 key, 4 slots, 2 (last-two-groups), 1024 (2 blocks x 2 heads x 256)].

    Additive mask applied to the last two block-groups of each slot's
    schedule; covers both the diagonal (partially causal) group and, for
    the parity whose exact schedule is shorter, a fully-masked group.
    """
    m = np.zeros((P, 4, 2, 1024), np.float32)
    for s in range(4):
        t = TILE_ASSIGN[parity][s]
        ng = NB_SCHED[s] // 2
        row_g = t * TILE_R + np.arange(TILE_R)
        for jj in range(2):  # which of the last two groups
            g = ng - 2 + jj
            for j in range(2):  # block within group
                blk = 2 * g + j
                key_g = blk * KB + np.arange(KB)
                bad = key_g[:, None] > row_g[None, :]
                for h in range(2):
                    sl = m[:, s, jj, j * 512 + h * 256 : j * 512 + (h + 1) * 256]
                    sl[bad] = -1e30
    return m.astype(BF16)


def _build_nc():
    import concourse.tile as tile
    from concourse import bacc, mybir

    f32 = mybir.dt.float32
    bf16 = mybir.dt.bfloat16
    Exp = mybir.ActivationFunctionType.Exp
    Mult = mybir.AluOpType.mult
    Add = mybir.AluOpType.add

    nc = bacc.Bacc("TRN2", target_bir_lowering=False, debug=False)

    xTkv = nc.dram_tensor("xTkv", [D, S], bf16, kind="ExternalInput")
    xTq = nc.dram_tensor("xTq", [D, ROWS_PER_CORE], bf16, kind="ExternalInput")
    wq1 = nc.dram_tensor("wq1", [D, RQ], bf16, kind="ExternalInput")
    wq2 = nc.dram_tensor("wq2", [RQ, H * DK], bf16, kind="ExternalInput")
    wk1 = nc.dram_tensor("wk1", [D, RKV], bf16, kind="ExternalInput")
    wk2 = nc.dram_tensor("wk2", [RKV, HKV * DK], bf16, kind="ExternalInput")
    wv1 = nc.dram_tensor("wv1", [D, RKV], bf16, kind="ExternalInput")
    wv2 = nc.dram_tensor("wv2", [RKV, HKV * DK], bf16, kind="ExternalInput")
    wo = nc.dram_tensor("wo", [D, D], bf16, kind="ExternalInput")
    maskin = nc.dram_tensor("maskin", [P, 4, 2, 1024], bf16, kind="ExternalInput")
    ones_in = nc.dram_tensor("ones_in", [P, P], bf16, kind="ExternalInput")
    out = nc.dram_tensor("out", [ROWS_PER_CORE, D], f32, kind="ExternalOutput")

    xTkv_t = xTkv.rearrange("(dc p) s -> p dc s", p=P)  # [128, 16, 2048]
    xTq_t = xTq.rearrange("(dc p) r -> p dc r", p=P)  # [128, 16, 1024]
    wq1_t = wq1.rearrange("(dc p) r -> p dc r", p=P)  # [128, 16, 512]
    wk1_t = wk1.rearrange("(dc p) r -> p dc r", p=P)
    wv1_t = wv1.rearrange("(dc p) r -> p dc r", p=P)
    wq2_t = wq2.rearrange("(rc p) h -> p rc h", p=P)  # [128, 4, 2048]
    wk2_t = wk2.rearrange("(rc p) h -> p rc h", p=P)  # [128, 4, 512]
    wv2_t = wv2.rearrange("(rc p) h -> p rc h", p=P)
    wo_t = wo.rearrange("(hc p) o -> p hc o", p=P)  # [128, 16, 2048]

    with tile.TileContext(nc) as tc:
        with tc.tile_pool(name="keep", bufs=1) as keep:
            ones_sb = keep.tile([P, P], bf16)
            kT_sb = keep.tile([P, HKV, S], bf16)
            v_sb = keep.tile([P, S // P, HKV * DK], bf16)
            qT_all = keep.tile([P, H, ROWS_PER_CORE], bf16)

            with tc.tile_pool(name="kvw", bufs=1) as kvw:
                w1k_sb = kvw.tile([P, 16, RKV], bf16)
                w1v_sb = kvw.tile([P, 16, RKV], bf16)

                # ---------------- Phase 1: Q projection (resident) --------
                with (
                    tc.tile_pool(name="q_w", bufs=1) as q_w,
                    tc.tile_pool(name="q_x", bufs=1) as q_x,
                    tc.tile_pool(name="q_mid", bufs=1) as q_mid,
                    tc.tile_pool(name="q_ps", bufs=4, space="PSUM") as q_ps,
                ):
                    wq1_sb = q_w.tile([P, 16, RQ], bf16)
                    xtq_sb = q_x.tile([P, 16, ROWS_PER_CORE], bf16)
                    # fine-grained pieces so the first matmul starts early
                    for dq in range(4):
                        nc.sync.dma_start(
                            wq1_sb[:, dq * 4 : (dq + 1) * 4],
                            wq1_t[:, dq * 4 : (dq + 1) * 4],
                        )
                        nc.sync.dma_start(
                            xtq_sb[:, dq * 4 : (dq + 1) * 4],
                            xTq_t[:, dq * 4 : (dq + 1) * 4],
                        )
                    wq2_sb = q_w.tile([P, 4, H * DK], bf16)
                    nc.sync.dma_start(wq2_sb[:], wq2_t)
                    nc.sync.dma_start(ones_sb[:], ones_in[:])
                    # prefetch K/V first-stage weights during Q compute
                    for dq in range(4):
                        nc.sync.dma_start(
                            w1k_sb[:, dq * 4 : (dq + 1) * 4],
                            wk1_t[:, dq * 4 : (dq + 1) * 4],
                        )
                        nc.sync.dma_start(
                            w1v_sb[:, dq * 4 : (dq + 1) * 4],
                            wv1_t[:, dq * 4 : (dq + 1) * 4],
                        )

                    q1t = q_mid.tile([P, 4, ROWS_PER_CORE], bf16)
                    for cn in range(2):
                        for rc in range(4):
                            ps_q = q_ps.tile([P, 512], f32, tag="psq1")
                            for dc in range(16):
                                nc.tensor.matmul(
                                    ps_q[:],
                                    wq1_sb[:, dc, rc * P : (rc + 1) * P],
                                    xtq_sb[:, dc, cn * 512 : (cn + 1) * 512],
                                    start=(dc == 0),
                                    stop=(dc == 15),
                                )
                            nc.any.tensor_copy(
                                q1t[:, rc, cn * 512 : (cn + 1) * 512], ps_q[:]
                            )
                    for h in range(H):
                        for cn in range(2):
                            ps_qT = q_ps.tile([P, 512], f32, tag="psq2")
                            for rc in range(4):
                                nc.tensor.matmul(
                                    ps_qT[:],
                                    wq2_sb[:, rc, h * P : (h + 1) * P],
                                    q1t[:, rc, cn * 512 : (cn + 1) * 512],
                                    start=(rc == 0),
                                    stop=(rc == 3),
                                )
                            nc.any.tensor_copy(
                                qT_all[:, h, cn * 512 : (cn + 1) * 512], ps_qT[:]
                            )

                with tc.tile_pool(name="attnkeep", bufs=1) as attnkeep:
                    attn_all = attnkeep.tile([P, 4, H // 2, 512], bf16)
                    mask_sb = attnkeep.tile([P, 4, 2, 1024], bf16)
                    nc.sync.dma_start(mask_sb[:], maskin[:])

                    # ------------- Phase 2: K and V projections -----------
                    with (
                        tc.tile_pool(name="kv_w2", bufs=1) as kv_w2,
                        tc.tile_pool(name="kv_x", bufs=2) as kv_x,
                        tc.tile_pool(name="kv_mid", bufs=1) as kv_mid,
                        tc.tile_pool(name="kv_ps", bufs=4, space="PSUM") as kv_ps,
                    ):
                        w2k_sb = kv_w2.tile([P, 4, HKV * DK], bf16)
                        nc.sync.dma_start(w2k_sb[:], wk2_t)
                        w2v_sb = kv_w2.tile([P, 4, HKV * DK], bf16)
                        nc.sync.dma_start(w2v_sb[:], wv2_t)

                        for tcn in range(4):  # token 512-chunks
                            xt = kv_x.tile([P, 16, 512], bf16, tag="xt")
                            for dq in range(4):
                                nc.sync.dma_start(
                                    xt[:, dq * 4 : (dq + 1) * 4],
                                    xTkv_t[
                                        :,
                                        dq * 4 : (dq + 1) * 4,
                                        tcn * 512 : (tcn + 1) * 512,
                                    ],
                                )
                            midk = kv_mid.tile([P, 4, 512], bf16, tag="midk")
                            midv = kv_mid.tile([P, 4, 512], bf16, tag="midv")
                            for rc in range(4):
                                ps_1 = kv_ps.tile([P, 512], f32, tag="ps1")
                                for dc in range(16):
                                    nc.tensor.matmul(
                                        ps_1[:],
                                        w1k_sb[:, dc, rc * P : (rc + 1) * P],
                                        xt[:, dc],
                                        start=(dc == 0),
                                        stop=(dc == 15),
                                    )
                                nc.any.tensor_copy(midk[:, rc], ps_1[:])
                            for rc in range(4):
                                ps_1 = kv_ps.tile([P, 512], f32, tag="ps1")
                                for dc in range(16):
                                    nc.tensor.matmul(
                                        ps_1[:],
                                        w1v_sb[:, dc, rc * P : (rc + 1) * P],
                                        xt[:, dc],
                                        start=(dc == 0),
                                        stop=(dc == 15),
                                    )
                                nc.any.tensor_copy(midv[:, rc], ps_1[:])
                            for hc in range(HKV):
                                ps_2 = kv_ps.tile([P, 512], f32, tag="ps2")
                                for rc in range(4):
                                    nc.tensor.matmul(
                                        ps_2[:],
                                        w2k_sb[:, rc, hc * P : (hc + 1) * P],
                                        midk[:, rc],
                                        start=(rc == 0),
                                        stop=(rc == 3),
                                    )
                                nc.any.tensor_copy(
                                    kT_sb[:, hc, tcn * 512 : (tcn + 1) * 512],
                                    ps_2[:],
                                )
                            for i in range(4):
                                ps_2 = kv_ps.tile([P, 512], f32, tag="ps2")
                                for rc in range(4):
                                    nc.tensor.matmul(
                                        ps_2[:],
                                        midv[:, rc, i * P : (i + 1) * P],
                                        w2v_sb[:, rc],
                                        start=(rc == 0),
                                        stop=(rc == 3),
                                    )
                                nc.any.tensor_copy(v_sb[:, tcn * 4 + i], ps_2[:])

                    # ------------- Phase 3: attention + Phase 4: Wo -------
                    with tc.tile_pool(name="wo_w", bufs=2) as wo_w:
                        wo_tiles = {}

                        def fetch_wo(oc):
                            t = wo_w.tile([P, 16, 512], bf16, tag="woc")
                            nc.sync.dma_start(t[:], wo_t[:, :, oc * 512 : (oc + 1) * 512])
                            wo_tiles[oc] = t

                        with (
                            tc.tile_pool(name="at_e", bufs=3) as at_e,
                            tc.tile_pool(name="at_rec", bufs=2) as at_rec,
                            tc.tile_pool(name="at_ps", bufs=2, space="PSUM") as at_ps,
                            tc.tile_pool(name="at_ps_acc", bufs=2, space="PSUM") as at_ps_acc,
                            tc.tile_pool(name="at_ps_sum", bufs=2, space="PSUM") as at_ps_sum,
                        ):
                            for si, s in enumerate((0, 1, 2, 3)):
                                if si == 2:
                                    fetch_wo(0)
                                if si == 3:
                                    fetch_wo(1)
                                ng = NB_SCHED[s] // 2
                                for hp in range(H // 2):
                                    h0 = 2 * hp
                                    kvh = h0 // GROUP
                                    ps_at = at_ps_acc.tile([P, 512], f32, tag="at")
                                    ps_sum = at_ps_sum.tile([P, 512], f32, tag="sum")

                                    def emit_qk(g, sc_tiles):
                                        ps_sc = at_ps.tile([P, 1024], f32, tag="sc")
                                        for j in range(2):
                                            nc.tensor.matmul(
                                                ps_sc[:, j * 512 : (j + 1) * 512],
                                                kT_sb[
                                                    :,
                                                    kvh,
                                                    (2 * g + j) * KB : (2 * g + j + 1) * KB,
                                                ],
                                                qT_all[
                                                    :,
                                                    h0 : h0 + 2,
                                                    s * TILE_R : (s + 1) * TILE_R,
                                                ],
                                                start=True,
                                                stop=True,
                                            )
                                        sc_tiles[g] = ps_sc

                                    sc_tiles = {}
                                    emit_qk(0, sc_tiles)
                                    for g in range(ng):
                                        if g + 1 < ng:
                                            emit_qk(g + 1, sc_tiles)
                                        ps_sc = sc_tiles.pop(g)
                                        if g >= ng - 2:
                                            nc.any.tensor_tensor(
                                                ps_sc[:],
                                                ps_sc[:],
                                                mask_sb[:, s, g - (ng - 2)],
                                                Add,
                                            )
                                        e_sb = at_e.tile([P, 1024], bf16, tag="e")
                                        nc.scalar.activation(e_sb[:], ps_sc[:], Exp)
                                        for j in range(2):
                                            b = 2 * g + j
                                            nc.tensor.matmul(
                                                ps_at[:],
                                                v_sb[:, b, kvh * DK : (kvh + 1) * DK],
                                                e_sb[:, j * 512 : (j + 1) * 512],
                                                start=(g == 0 and j == 0),
                                                stop=(g == ng - 1 and j == 1),
                                            )
                                        for j in range(2):
                                            nc.tensor.matmul(
                                                ps_sum[:],
                                                ones_sb[:],
                                                e_sb[:, j * 512 : (j + 1) * 512],
                                                start=(g == 0 and j == 0),
                                                stop=(g == ng - 1 and j == 1),
                                            )
                                    rec_sb = at_rec.tile([P, 512], f32, tag="rec")
                                    nc.vector.reciprocal_approx_fast(
                                        out=rec_sb[:], in_=ps_sum[:]
                                    )
                                    nc.any.tensor_tensor(
                                        attn_all[:, s, hp], ps_at[:], rec_sb[:], Mult
                                    )

                        # ---- Phase 4: Wo ----
                        with (
                            tc.tile_pool(name="wo_out", bufs=3) as wo_out,
                            tc.tile_pool(name="wo_ps", bufs=3, space="PSUM") as wo_ps,
                        ):
                            for oc in range(4):
                                if oc not in wo_tiles:
                                    fetch_wo(oc)
                                wo_sb = wo_tiles[oc]
                                for rc in range(8):
                                    s, half = rc // 2, rc % 2
                                    ps_o = wo_ps.tile([P, 512], f32, tag="o")
                                    for hc in range(16):
                                        nc.tensor.matmul(
                                            ps_o[:],
                                            attn_all[
                                                :,
                                                s,
                                                hc // 2,
                                                (hc % 2) * 256
                                                + half * P : (hc % 2) * 256
                                                + (half + 1) * P,
                                            ],
                                            wo_sb[:, hc],
                                            start=(hc == 0),
                                            stop=(hc == 15),
                                        )
                                    o_sb = wo_out.tile([P, 512], f32, tag="osb")
                                    nc.any.tensor_copy(o_sb[:], ps_o[:])
                                    nc.sync.dma_start(
                                        out[
                                            rc * P : (rc + 1) * P,
                                            oc * 512 : (oc + 1) * 512,
                                        ],
                                        o_sb[:],
                                    )

    nc.finalize()
    return nc


def kernel(x, Wq1, Wq2, Wk1, Wk2, Wv1, Wv2, Wo):
    global LAST_RESULT
    from concourse.bass_utils import run_bass_kernel_spmd

    x = np.asarray(x, dtype=np.float32)
    Wq1 = np.asarray(Wq1, dtype=np.float32)
    Wq2 = np.asarray(Wq2, dtype=np.float32)
    Wk1 = np.asarray(Wk1, dtype=np.float32)
    Wk2 = np.asarray(Wk2, dtype=np.float32)
    Wv1 = np.asarray(Wv1, dtype=np.float32)
    Wv2 = np.asarray(Wv2, dtype=np.float32)
    Wo = np.asarray(Wo, dtype=np.float32)

    if "nc" not in _CACHE:
        _CACHE["nc"] = _build_nc()
    nc = _CACHE["nc"]

    wq1_bf = Wq1.astype(BF16)
    wq2_bf = (Wq2 / np.sqrt(DK)).astype(BF16)
    wk1_bf = Wk1.astype(BF16)
    wk2_bf = Wk2.astype(BF16)
    wv1_bf = Wv1.astype(BF16)
    wv2_bf = Wv2.astype(BF16)
    wo_bf = Wo.astype(BF16)
    masks = {p: _make_mask(p) for p in range(2)}
    rows = {p: _rows_sched(p) for p in range(2)}
    ones_np = np.ones((P, P), BF16)

    xT_bf = {}
    for batch in range(B):
        xT_bf[batch] = np.ascontiguousarray(x[batch].T).astype(BF16)

    in_maps = []
    for core in range(8):
        batch, parity = core // 2, core % 2
        xT = xT_bf[batch]
        in_maps.append(
            {
                "xTkv": xT,
                "xTq": np.ascontiguousarray(xT[:, rows[parity]]),
                "wq1": wq1_bf,
                "wq2": wq2_bf,
                "wk1": wk1_bf,
                "wk2": wk2_bf,
                "wv1": wv1_bf,
                "wv2": wv2_bf,
                "wo": wo_bf,
                "maskin": masks[parity],
                "ones_in": ones_np,
            }
        )

    res = run_bass_kernel_spmd(nc, in_maps, core_ids=list(range(8)), trace=TRACE)
    LAST_RESULT = res

    out_full = np.empty((B, S, D), np.float32)
    for core in range(8):
        batch, parity = core // 2, core % 2
        out_full[batch][rows[parity]] = res.results[core]["out"]
    return out_full


# revision 11
# speedup vs baseline: 1.7275x; 1.1865x over previous
"""TRN2 Bass kernel for nn_CoreAttention_34875134444341 (v4).

Strategy (8 NeuronCores, pairwise AllGather):
  - Data-parallel over batch (4) x causal-balanced query-row split (2).
  - Each core computes K/V projections ONLY for its own 1024 query
    tokens (straight from the resident xTq), then a 2-core AllGather
    (replica groups [0,1],[2,3],[4,5],[6,7]) exchanges K/V shards.
    The collective runs concurrently with the Q projection phase.
  - K/V storage is rank-major ([rank0 shard | rank1 shard], each in
    schedule order), so the program is identical on both parities;
    every parity difference lives in the mask input.
  - 128-query-row tiles, zig-zag assigned so both parities run the
    same per-slot group counts NG=[8,7,6,5,4,3,2,1]. Attention
    processes one GQA quad (4 heads sharing a kv head) x 128 rows per
    N=512 matmul; one key block from each rank's shard per group
    (exp on [128,1024]).
  - Causal masks enter PSUM via an identity-stationary matmul opening
    the last group's accumulation; softmax denominators via a
    ones[128,128] stationary matmul (broadcast across partitions) +
    reciprocal_approx_fast + multiply.
  - All matmul operands bf16 (FWL, half DMA/SBUF); PSUM fp32.
"""

import sys

sys.path.insert(0, "/opt/trn_rl_repo")

import numpy as np
import ml_dtypes

B, S, D = 4, 2048, 2048
H, HKV, DK = 16, 4, 128
RQ = RKV = 512
GROUP = H // HKV
P = 128

TILE_R = 128  # query rows per slot
KB = 128  # keys per block
NG = [8, 7, 6, 5, 4, 3, 2, 1]  # key-block pairs per slot (both parities)
TILE_ASSIGN = {
    0: [15, 12, 11, 8, 7, 4, 3, 0],
    1: [14, 13, 10, 9, 6, 5, 2, 1],
}  # slot -> query tile (descending); tiles double as this core's kv shard

ROWS_PER_CORE = 8 * TILE_R  # 1024
SHARD = 8 * TILE_R  # kv tokens per core
CC_K = HKV * SHARD  # kT part of the collective shard (per partition row)
CC_N = CC_K + 8 * 512  # + v part

_CACHE = {}
TRACE = False
LAST_RESULT = None

BF16 = ml_dtypes.bfloat16


def _rows_sched(parity):
    return np.concatenate(
        [np.arange(t * TILE_R, (t + 1) * TILE_R) for t in TILE_ASSIGN[parity]]
    )


def _make_mask(parity):
    """[128 key, 8 slots, 2 banks, 512 (4 heads x 128 rows)].

    Additive mask for the LAST group of each slot. Bank 0 reads rank0's
    shard, bank 1 rank1's. The slot's own diagonal tile is always the
    last block of the OWN rank's prefix; the other rank's prefix is
    padded by one fully-masked block on the slots where its exact
    causal need is one block short of the uniform schedule.
    """
    m = np.zeros((P, 8, 2, 512), np.float32)
    diag = (np.arange(P)[:, None] > np.arange(TILE_R)[None, :]).astype(np.float32)
    diag4 = np.tile(diag * -1e30, (1, 4))  # same for each of the 4 heads
    own, other = parity, 1 - parity
    own_tiles = sorted(TILE_ASSIGN[own])
    other_tiles = sorted(TILE_ASSIGN[other])
    for s in range(8):
        t = TILE_ASSIGN[parity][s]
        ng = NG[s]
        assert sum(1 for x in own_tiles if x <= t) == ng
        m[:, s, own] = diag4
        exact_other = sum(1 for x in other_tiles if x <= t)
        assert exact_other in (ng, ng - 1)
        if exact_other == ng - 1:
            m[:, s, other] = -1e30
    return m.astype(BF16)


def _build_nc():
    import concourse.tile as tile
    from concourse import bacc, mybir

    f32 = mybir.dt.float32
    bf16 = mybir.dt.bfloat16
    Exp = mybir.ActivationFunctionType.Exp
    Mult = mybir.AluOpType.mult

    nc = bacc.Bacc("TRN2", target_bir_lowering=False, debug=False, num_devices=8)

    xTq = nc.dram_tensor("xTq", [D, ROWS_PER_CORE], bf16, kind="ExternalInput")
    wq1 = nc.dram_tensor("wq1", [D, RQ], bf16, kind="ExternalInput")
    wq2 = nc.dram_tensor("wq2", [RQ, H * DK], bf16, kind="ExternalInput")
    wk1 = nc.dram_tensor("wk1", [D, RKV], bf16, kind="ExternalInput")
    wk2 = nc.dram_tensor("wk2", [RKV, HKV * DK], bf16, kind="ExternalInput")
    wv1 = nc.dram_tensor("wv1", [D, RKV], bf16, kind="ExternalInput")
    wv2 = nc.dram_tensor("wv2", [RKV, HKV * DK], bf16, kind="ExternalInput")
    wo = nc.dram_tensor("wo", [D, D], bf16, kind="ExternalInput")
    maskin = nc.dram_tensor("maskin", [P, 8, 2, 512], bf16, kind="ExternalInput")
    ones_in = nc.dram_tensor("ones_in", [P, P], bf16, kind="ExternalInput")
    ident_in = nc.dram_tensor("ident_in", [P, P], bf16, kind="ExternalInput")
    out = nc.dram_tensor("out", [ROWS_PER_CORE, D], f32, kind="ExternalOutput")

    xTq_t = xTq.rearrange("(dc p) r -> p dc r", p=P)  # [128, 16, 1024]
    wq1_t = wq1.rearrange("(dc p) r -> p dc r", p=P)  # [128, 16, 512]
    wk1_t = wk1.rearrange("(dc p) r -> p dc r", p=P)
    wv1_t = wv1.rearrange("(dc p) r -> p dc r", p=P)
    wq2_t = wq2.rearrange("(rc p) h -> p rc h", p=P)  # [128, 4, 2048]
    wk2_t = wk2.rearrange("(rc p) h -> p rc h", p=P)  # [128, 4, 512]
    wv2_t = wv2.rearrange("(rc p) h -> p rc h", p=P)
    wo_t = wo.rearrange("(hc p) o -> p hc o", p=P)  # [128, 16, 2048]

    with tile.TileContext(nc) as tc:
        with (
            tc.tile_pool(name="keep", bufs=1) as keep,
            tc.tile_pool(name="cc_dram", bufs=1, space="DRAM") as cc_dram,
        ):
            ones_sb = keep.tile([P, P], bf16)
            ident_sb = keep.tile([P, P], bf16)
            kT_sb = keep.tile([P, HKV, S], bf16)
            v_sb = keep.tile([P, S // P, HKV * DK], bf16)
            qT_all = keep.tile([P, H, ROWS_PER_CORE], bf16)

            cc_in = cc_dram.tile([P, CC_N], bf16)
            cc_out = cc_dram.tile([2, P, CC_N], bf16)

            with tc.tile_pool(name="phase_a", bufs=1) as phase_a:
                xtq_ts = [
                    phase_a.tile([P, 4, ROWS_PER_CORE], bf16, name=f"xtq_p{dq}")
                    for dq in range(4)
                ]
                w1k_sb = phase_a.tile([P, 16, RKV], bf16)
                w1v_sb = phase_a.tile([P, 16, RKV], bf16)
                wq1_ts = [
                    phase_a.tile([P, 4, RQ], bf16, name=f"wq1_p{dq}")
                    for dq in range(4)
                ]
                wq2_sb = phase_a.tile([P, 4, H * DK], bf16)

                # critical-path DMAs first (KV-half runs before Q)
                for dq in range(4):
                    nc.sync.dma_start(xtq_ts[dq][:], xTq_t[:, dq * 4 : (dq + 1) * 4])
                    nc.sync.dma_start(
                        w1k_sb[:, dq * 4 : (dq + 1) * 4],
                        wk1_t[:, dq * 4 : (dq + 1) * 4],
                    )
                for dq in range(4):
                    nc.sync.dma_start(
                        w1v_sb[:, dq * 4 : (dq + 1) * 4],
                        wv1_t[:, dq * 4 : (dq + 1) * 4],
                    )
                for dq in range(4):
                    nc.sync.dma_start(wq1_ts[dq][:], wq1_t[:, dq * 4 : (dq + 1) * 4])
                nc.sync.dma_start(wq2_sb[:], wq2_t)
                nc.sync.dma_start(ones_sb[:], ones_in[:])
                nc.sync.dma_start(ident_sb[:], ident_in[:])

                # ------- Phase A1: K/V projections for OWN tokens ---------
                with (
                    tc.tile_pool(name="kv_w2", bufs=1) as kv_w2,
                    tc.tile_pool(name="kv_mid", bufs=1) as kv_mid,
                    tc.tile_pool(name="kv_bounce", bufs=4) as kv_bounce,
                    tc.tile_pool(name="kv_ps", bufs=4, space="PSUM") as kv_ps,
                ):
                    w2k_sb = kv_w2.tile([P, 4, HKV * DK], bf16)
                    nc.sync.dma_start(w2k_sb[:], wk2_t)
                    w2v_sb = kv_w2.tile([P, 4, HKV * DK], bf16)
                    nc.sync.dma_start(w2v_sb[:], wv2_t)

                    midk = kv_mid.tile([P, 4, ROWS_PER_CORE], bf16)
                    midv = kv_mid.tile([P, 4, ROWS_PER_CORE], bf16)
                    # piece-major accumulation for the first chunk so the
                    # first matmul only needs DMA piece 0
                    ps_ks = [
                        kv_ps.tile([P, 512], f32, tag="ps1", name=f"ps_k_{rc}")
                        for rc in range(4)
                    ]
                    for dq in range(4):
                        for rc in range(4):
                            for dc in range(4 * dq, 4 * dq + 4):
                                nc.tensor.matmul(
                                    ps_ks[rc][:],
                                    w1k_sb[:, dc, rc * P : (rc + 1) * P],
                                    xtq_ts[dc // 4][:, dc % 4, 0:512],
                                    start=(dc == 0),
                                    stop=(dc == 15),
                                )
                    for rc in range(4):
                        nc.any.tensor_copy(midk[:, rc, 0:512], ps_ks[rc][:])
                    for which in range(3):  # V cn0, K cn1, V cn1
                        w1_sb = w1v_sb if which != 1 else w1k_sb
                        mid = midv if which != 1 else midk
                        cn = 0 if which == 0 else 1
                        for rc in range(4):
                            ps_1 = kv_ps.tile([P, 512], f32, tag="ps1")
                            for dc in range(16):
                                nc.tensor.matmul(
                                    ps_1[:],
                                    w1_sb[:, dc, rc * P : (rc + 1) * P],
                                    xtq_ts[dc // 4][:, dc % 4, cn * 512 : cn * 512 + 512],
                                    start=(dc == 0),
                                    stop=(dc == 15),
                                )
                            nc.any.tensor_copy(
                                mid[:, rc, cn * 512 : cn * 512 + 512], ps_1[:]
                            )
                    # second-stage projections -> collective shard (DRAM)
                    for cn in range(2):
                        for hc in range(HKV):
                            ps_2 = kv_ps.tile([P, 512], f32, tag="ps2")
                            for rc in range(4):
                                nc.tensor.matmul(
                                    ps_2[:],
                                    w2k_sb[:, rc, hc * P : (hc + 1) * P],
                                    midk[:, rc, cn * 512 : cn * 512 + 512],
                                    start=(rc == 0),
                                    stop=(rc == 3),
                                )
                            kb = kv_bounce.tile([P, 512], bf16, tag="kb")
                            nc.any.tensor_copy(kb[:], ps_2[:])
                            nc.sync.dma_start(
                                cc_in[:, hc * SHARD + cn * 512 : hc * SHARD + cn * 512 + 512],
                                kb[:],
                            )
                        for i in range(4):
                            ps_2 = kv_ps.tile([P, 512], f32, tag="ps2")
                            for rc in range(4):
                                nc.tensor.matmul(
                                    ps_2[:],
                                    midv[:, rc, cn * 512 + i * P : cn * 512 + (i + 1) * P],
                                    w2v_sb[:, rc],
                                    start=(rc == 0),
                                    stop=(rc == 3),
                                )
                            vb = kv_bounce.tile([P, 512], bf16, tag="vb")
                            nc.any.tensor_copy(vb[:], ps_2[:])
                            nc.sync.dma_start(
                                cc_in[
                                    :,
                                    CC_K + (cn * 4 + i) * 512 : CC_K + (cn * 4 + i + 1) * 512,
                                ],
                                vb[:],
                            )

                # ------- AllGather kicks off; Q projection hides it -------
                nc.gpsimd.collective_compute(
                    "AllGather",
                    mybir.AluOpType.bypass,
                    replica_groups=[[0, 1], [2, 3], [4, 5], [6, 7]],
                    ins=[cc_in[:].opt()],
                    outs=[cc_out[:].opt()],
                )

                # ------- Phase A2: Q projection (resident) ----------------
                with (
                    tc.tile_pool(name="q_mid", bufs=1) as q_mid,
                    tc.tile_pool(name="q_ps", bufs=4, space="PSUM") as q_ps,
                ):
                    q1t = q_mid.tile([P, 4, ROWS_PER_CORE], bf16)
                    for cn in range(2):
                        for rc in range(4):
                            ps_q = q_ps.tile([P, 512], f32, tag="psq1")
                            for dc in range(16):
                                nc.tensor.matmul(
                                    ps_q[:],
                                    wq1_ts[dc // 4][:, dc % 4, rc * P : (rc + 1) * P],
                                    xtq_ts[dc // 4][:, dc % 4, cn * 512 : cn * 512 + 512],
                                    start=(dc == 0),
                                    stop=(dc == 15),
                                )
                            nc.any.tensor_copy(
                                q1t[:, rc, cn * 512 : cn * 512 + 512], ps_q[:]
                            )
                    for h in range(H):
                        for cn in range(2):
                            ps_qT = q_ps.tile([P, 512], f32, tag="psq2")
                            for rc in range(4):
                                nc.tensor.matmul(
                                    ps_qT[:],
                                    wq2_sb[:, rc, h * P : (h + 1) * P],
                                    q1t[:, rc, cn * 512 : cn * 512 + 512],
                                    start=(rc == 0),
                                    stop=(rc == 3),
                                )
                            nc.any.tensor_copy(
                                qT_all[:, h, cn * 512 : cn * 512 + 512], ps_qT[:]
                            )

            # ------- load gathered K/V shards into SBUF -------------------
            with tc.tile_pool(name="attnkeep", bufs=1) as attnkeep:
                attn_all = attnkeep.tile([P, 8, HKV, 512], bf16)
                mask_sb = attnkeep.tile([P, 8, 2, 512], bf16)
                nc.sync.dma_start(mask_sb[:], maskin[:])
                for r in range(2):
                    for kvh in range(HKV):
                        nc.sync.dma_start(
                            kT_sb[:, kvh, r * SHARD : (r + 1) * SHARD],
                            cc_out[r, :, kvh * SHARD : (kvh + 1) * SHARD],
                        )
                    nc.sync.dma_start(
                        v_sb[:, r * 8 : (r + 1) * 8, :],
                        cc_out[r, :, CC_K:CC_N].rearrange(
                            "p (b c) -> p b c", b=8
                        ),
                    )

                # --------- Phase 3: attention + Phase 4: Wo ---------------
                with tc.tile_pool(name="wo_w", bufs=2) as wo_w:
                    wo_tiles = {}

                    def fetch_wo(oc):
                        t = wo_w.tile([P, 16, 512], bf16, tag="woc")
                        nc.sync.dma_start(t[:], wo_t[:, :, oc * 512 : (oc + 1) * 512])
                        wo_tiles[oc] = t

                    with (
                        tc.tile_pool(name="at_e", bufs=3) as at_e,
                        tc.tile_pool(name="at_rec", bufs=2) as at_rec,
                        tc.tile_pool(name="at_ps", bufs=2, space="PSUM") as at_ps,
                        tc.tile_pool(name="at_ps_acc", bufs=2, space="PSUM") as at_ps_acc,
                        tc.tile_pool(name="at_ps_sum", bufs=2, space="PSUM") as at_ps_sum,
                    ):
                        for s in range(8):
                            if s == 6:
                                fetch_wo(0)
                            if s == 7:
                                fetch_wo(1)
                            ng = NG[s]
                            for kvh in range(HKV):
                                h0 = 4 * kvh
                                ps_at = at_ps_acc.tile([P, 512], f32, tag="at")
                                ps_sum = at_ps_sum.tile([P, 512], f32, tag="sum")

                                def emit_qk(g, sc_tiles):
                                    masked = g == ng - 1
                                    ps_sc = at_ps.tile([P, 1024], f32, tag="sc")
                                    for j in range(2):
                                        # bank j reads rank j's shard,
                                        # storage position 7-g (shards are
                                        # schedule-ordered = tile-descending)
                                        pos = 7 - g
                                        if masked:
                                            nc.tensor.matmul(
                                                ps_sc[:, j * 512 : (j + 1) * 512],
                                                ident_sb[:],
                                                mask_sb[:, s, j],
                                                start=True,
                                                stop=False,
                                            )
                                        nc.tensor.matmul(
                                            ps_sc[:, j * 512 : (j + 1) * 512],
                                            kT_sb[
                                                :,
                                                kvh,
                                                j * SHARD
                                                + pos * KB : j * SHARD
                                                + (pos + 1) * KB,
                                            ],
                                            qT_all[
                                                :,
                                                h0 : h0 + 4,
                                                s * TILE_R : (s + 1) * TILE_R,
                                            ],
                                            start=not masked,
                                            stop=True,
                                        )
                                    sc_tiles[g] = ps_sc

                                sc_tiles = {}
                                emit_qk(0, sc_tiles)
                                for g in range(ng):
                                    if g + 1 < ng:
                                        emit_qk(g + 1, sc_tiles)
                                    ps_sc = sc_tiles.pop(g)
                                    e_sb = at_e.tile([P, 1024], bf16, tag="e")
                                    nc.scalar.activation(e_sb[:], ps_sc[:], Exp)
                                    for j in range(2):
                                        pos = 7 - g
                                        nc.tensor.matmul(
                                            ps_at[:],
                                            v_sb[
                                                :,
                                                j * 8 + pos,
                                                kvh * DK : (kvh + 1) * DK,
                                            ],
                                            e_sb[:, j * 512 : (j + 1) * 512],
                                            start=(g == 0 and j == 0),
                                            stop=(g == ng - 1 and j == 1),
                                        )
                                    for j in range(2):
                                        nc.tensor.matmul(
                                            ps_sum[:],
                                            ones_sb[:],
                                            e_sb[:, j * 512 : (j + 1) * 512],
                                            start=(g == 0 and j == 0),
                                            stop=(g == ng - 1 and j == 1),
                                        )
                                rec_sb = at_rec.tile([P, 512], f32, tag="rec")
                                nc.vector.reciprocal_approx_fast(
                                    out=rec_sb[:], in_=ps_sum[:]
                                )
                                nc.any.tensor_tensor(
                                    attn_all[:, s, kvh], ps_at[:], rec_sb[:], Mult
                                )

                    # ---- Phase 4: Wo ----
                    with (
                        tc.tile_pool(name="wo_out", bufs=3) as wo_out,
                        tc.tile_pool(name="wo_ps", bufs=3, space="PSUM") as wo_ps,
                    ):
                        for oc in range(4):
                            if oc not in wo_tiles:
                                fetch_wo(oc)
                            wo_sb = wo_tiles[oc]
                            for rc in range(8):
                                ps_o = wo_ps.tile([P, 512], f32, tag="o")
                                for hc in range(16):
                                    nc.tensor.matmul(
                                        ps_o[:],
                                        attn_all[
                                            :,
                                            rc,
                                            hc // 4,
                                            (hc % 4) * P : (hc % 4 + 1) * P,
                                        ],
                                        wo_sb[:, hc],
                                        start=(hc == 0),
                                        stop=(hc == 15),
                                    )
                                o_sb = wo_out.tile([P, 512], f32, tag="osb")
                                nc.any.tensor_copy(o_sb[:], ps_o[:])
                                nc.sync.dma_start(
                                    out[
                                        rc * P : (rc + 1) * P,
                                        oc * 512 : (oc + 1) * 512,
                                    ],
                                    o_sb[:],
                                )

    nc.finalize()
    return nc


def kernel(x, Wq1, Wq2, Wk1, Wk2, Wv1, Wv2, Wo):
    global LAST_RESULT
    from concourse.bass_utils import run_bass_kernel_spmd

    x = np.asarray(x, dtype=np.float32)
    Wq1 = np.asarray(Wq1, dtype=np.float32)
    Wq2 = np.asarray(Wq2, dtype=np.float32)
    Wk1 = np.asarray(Wk1, dtype=np.float32)
    Wk2 = np.asarray(Wk2, dtype=np.float32)
    Wv1 = np.asarray(Wv1, dtype=np.float32)
    Wv2 = np.asarray(Wv2, dtype=np.float32)
    Wo = np.asarray(Wo, dtype=np.float32)

    if "nc" not in _CACHE:
        _CACHE["nc"] = _build_nc()
    nc = _CACHE["nc"]

    wq1_bf = Wq1.astype(BF16)
    wq2_bf = (Wq2 / np.sqrt(DK)).astype(BF16)
    wk1_bf = Wk1.astype(BF16)
    wk2_bf = Wk2.astype(BF16)
    wv1_bf = Wv1.astype(BF16)
    wv2_bf = Wv2.astype(BF16)
    wo_bf = Wo.astype(BF16)
    masks = {p: _make_mask(p) for p in range(2)}
    rows = {p: _rows_sched(p) for p in range(2)}
    ones_np = np.ones((P, P), BF16)
    ident_np = np.eye(P, dtype=np.float32).astype(BF16)

    xT_bf = {}
    for batch in range(B):
        xT_bf[batch] = np.ascontiguousarray(x[batch].T).astype(BF16)

    in_maps = []
    for core in range(8):
        batch, parity = core // 2, core % 2
        xT = xT_bf[batch]
        in_maps.append(
            {
                "xTq": np.ascontiguousarray(xT[:, rows[parity]]),
                "wq1": wq1_bf,
                "wq2": wq2_bf,
                "wk1": wk1_bf,
                "wk2": wk2_bf,
                "wv1": wv1_bf,
                "wv2": wv2_bf,
                "wo": wo_bf,
                "maskin": masks[parity],
                "ones_in": ones_np,
                "ident_in": ident_np,
            }
        )

    res = run_bass_kernel_spmd(nc, in_maps, core_ids=list(range(8)), trace=TRACE)
    LAST_RESULT = res

    out_full = np.empty((B, S, D), np.float32)
    for core in range(8):
        batch, parity = core // 2, core % 2
        out_full[batch][rows[parity]] = res.results[core]["out"]
    return out_full


# revision 13
# speedup vs baseline: 1.7809x; 1.0309x over previous
"""TRN2 Bass kernel for nn_CoreAttention_34875134444341 (v4).

Strategy (8 NeuronCores, pairwise AllGather):
  - Data-parallel over batch (4) x causal-balanced query-row split (2).
  - Each core computes K/V projections ONLY for its own 1024 query
    tokens (straight from the resident xTq), then a 2-core AllGather
    (replica groups [0,1],[2,3],[4,5],[6,7]) exchanges K/V shards.
    The collective runs concurrently with the Q projection phase.
  - K/V storage is rank-major ([rank0 shard | rank1 shard], each in
    schedule order), so the program is identical on both parities;
    every parity difference lives in the mask input.
  - 128-query-row tiles, zig-zag assigned so both parities run the
    same per-slot group counts NG=[8,7,6,5,4,3,2,1]. Attention
    processes one GQA quad (4 heads sharing a kv head) x 128 rows per
    N=512 matmul; one key block from each rank's shard per group
    (exp on [128,1024]).
  - Causal masks enter PSUM via an identity-stationary matmul opening
    the last group's accumulation; softmax denominators via a
    ones[128,128] stationary matmul (broadcast across partitions) +
    reciprocal_approx_fast + multiply.
  - All matmul operands bf16 (FWL, half DMA/SBUF); PSUM fp32.
"""

import sys

sys.path.insert(0, "/opt/trn_rl_repo")

import numpy as np
import ml_dtypes

B, S, D = 4, 2048, 2048
H, HKV, DK = 16, 4, 128
RQ = RKV = 512
GROUP = H // HKV
P = 128

TILE_R = 128  # query rows per slot
KB = 128  # keys per block
NG = [8, 7, 6, 5, 4, 3, 2, 1]  # key-block pairs per slot (both parities)
TILE_ASSIGN = {
    0: [15, 12, 11, 8, 7, 4, 3, 0],
    1: [14, 13, 10, 9, 6, 5, 2, 1],
}  # slot -> query tile (descending); tiles double as this core's kv shard

ROWS_PER_CORE = 8 * TILE_R  # 1024
SHARD = 8 * TILE_R  # kv tokens per core
CC_K = HKV * SHARD  # kT part of the collective shard (per partition row)
CC_N = CC_K + 8 * 512  # + v part

_CACHE = {}
TRACE = False
LAST_RESULT = None

BF16 = ml_dtypes.bfloat16


def _rows_sched(parity):
    return np.concatenate(
        [np.arange(t * TILE_R, (t + 1) * TILE_R) for t in TILE_ASSIGN[parity]]
    )


def _make_mask(parity):
    """[128 key, 8 slots, 2 banks, 512 (4 heads x 128 rows)].

    Additive mask for the LAST group of each slot. Bank 0 reads rank0's
    shard, bank 1 rank1's. The slot's own diagonal tile is always the
    last block of the OWN rank's prefix; the other rank's prefix is
    padded by one fully-masked block on the slots where its exact
    causal need is one block short of the uniform schedule.
    """
    m = np.zeros((P, 8, 2, 512), np.float32)
    diag = (np.arange(P)[:, None] > np.arange(TILE_R)[None, :]).astype(np.float32)
    diag4 = np.tile(diag * -1e30, (1, 4))  # same for each of the 4 heads
    own, other = parity, 1 - parity
    own_tiles = sorted(TILE_ASSIGN[own])
    other_tiles = sorted(TILE_ASSIGN[other])
    for s in range(8):
        t = TILE_ASSIGN[parity][s]
        ng = NG[s]
        assert sum(1 for x in own_tiles if x <= t) == ng
        m[:, s, own] = diag4
        exact_other = sum(1 for x in other_tiles if x <= t)
        assert exact_other in (ng, ng - 1)
        if exact_other == ng - 1:
            m[:, s, other] = -1e30
    return m.astype(BF16)


def _build_nc():
    import concourse.tile as tile
    from concourse import bacc, mybir

    f32 = mybir.dt.float32
    bf16 = mybir.dt.bfloat16
    Exp = mybir.ActivationFunctionType.Exp
    Mult = mybir.AluOpType.mult

    nc = bacc.Bacc("TRN2", target_bir_lowering=False, debug=False, num_devices=8)

    xTq = nc.dram_tensor("xTq", [D, ROWS_PER_CORE], bf16, kind="ExternalInput")
    wq1 = nc.dram_tensor("wq1", [D, RQ], bf16, kind="ExternalInput")
    wq2 = nc.dram_tensor("wq2", [RQ, H * DK], bf16, kind="ExternalInput")
    wk1 = nc.dram_tensor("wk1", [D, RKV], bf16, kind="ExternalInput")
    wk2 = nc.dram_tensor("wk2", [RKV, HKV * DK], bf16, kind="ExternalInput")
    wv1 = nc.dram_tensor("wv1", [D, RKV], bf16, kind="ExternalInput")
    wv2 = nc.dram_tensor("wv2", [RKV, HKV * DK], bf16, kind="ExternalInput")
    wo = nc.dram_tensor("wo", [D, D], bf16, kind="ExternalInput")
    maskin = nc.dram_tensor("maskin", [P, 8, 2, 512], bf16, kind="ExternalInput")
    ones_in = nc.dram_tensor("ones_in", [P, P], bf16, kind="ExternalInput")
    ident_in = nc.dram_tensor("ident_in", [P, P], bf16, kind="ExternalInput")
    out = nc.dram_tensor("out", [ROWS_PER_CORE, D], f32, kind="ExternalOutput")

    xTq_t = xTq.rearrange("(dc p) r -> p dc r", p=P)  # [128, 16, 1024]
    wq1_t = wq1.rearrange("(dc p) r -> p dc r", p=P)  # [128, 16, 512]
    wk1_t = wk1.rearrange("(dc p) r -> p dc r", p=P)
    wv1_t = wv1.rearrange("(dc p) r -> p dc r", p=P)
    wq2_t = wq2.rearrange("(rc p) h -> p rc h", p=P)  # [128, 4, 2048]
    wk2_t = wk2.rearrange("(rc p) h -> p rc h", p=P)  # [128, 4, 512]
    wv2_t = wv2.rearrange("(rc p) h -> p rc h", p=P)
    wo_t = wo.rearrange("(hc p) o -> p hc o", p=P)  # [128, 16, 2048]

    with tile.TileContext(nc) as tc:
        with (
            tc.tile_pool(name="keep", bufs=1) as keep,
            tc.tile_pool(name="cc_dram", bufs=1, space="DRAM") as cc_dram,
        ):
            ones_sb = keep.tile([P, P], bf16)
            ident_sb = keep.tile([P, P], bf16)
            kT_sb = keep.tile([P, HKV, S], bf16)
            v_sb = keep.tile([P, S // P, HKV * DK], bf16)
            qT_all = keep.tile([P, H, ROWS_PER_CORE], bf16)
            mask_sb = keep.tile([P, 8, 2, 512], bf16)

            cc_in = cc_dram.tile([P, CC_N], bf16)
            cc_out = cc_dram.tile([2, P, CC_N], bf16)

            with tc.tile_pool(name="phase_a", bufs=1) as phase_a:
                xtq_ts = [
                    phase_a.tile([P, 4, ROWS_PER_CORE], bf16, name=f"xtq_p{dq}")
                    for dq in range(4)
                ]
                w1k_sb = phase_a.tile([P, 16, RKV], bf16)
                w1v_sb = phase_a.tile([P, 16, RKV], bf16)
                wq1_ts = [
                    phase_a.tile([P, 4, RQ], bf16, name=f"wq1_p{dq}")
                    for dq in range(4)
                ]
                wq2_sb = phase_a.tile([P, 4, H * DK], bf16)

                # critical-path DMAs first (KV-half runs before Q)
                for dq in range(4):
                    nc.sync.dma_start(xtq_ts[dq][:], xTq_t[:, dq * 4 : (dq + 1) * 4])
                    nc.sync.dma_start(
                        w1k_sb[:, dq * 4 : (dq + 1) * 4],
                        wk1_t[:, dq * 4 : (dq + 1) * 4],
                    )
                for dq in range(4):
                    nc.sync.dma_start(
                        w1v_sb[:, dq * 4 : (dq + 1) * 4],
                        wv1_t[:, dq * 4 : (dq + 1) * 4],
                    )
                for dq in range(4):
                    nc.sync.dma_start(wq1_ts[dq][:], wq1_t[:, dq * 4 : (dq + 1) * 4])
                nc.sync.dma_start(wq2_sb[:], wq2_t)
                nc.sync.dma_start(ones_sb[:], ones_in[:])
                nc.sync.dma_start(ident_sb[:], ident_in[:])
                nc.sync.dma_start(mask_sb[:], maskin[:])

                # ------- Phase A1: K/V projections for OWN tokens ---------
                with (
                    tc.tile_pool(name="kv_w2", bufs=1) as kv_w2,
                    tc.tile_pool(name="kv_mid", bufs=1) as kv_mid,
                    tc.tile_pool(name="kv_bounce", bufs=3) as kv_bounce,
                    tc.tile_pool(name="kv_ps", bufs=4, space="PSUM") as kv_ps,
                ):
                    w2k_sb = kv_w2.tile([P, 4, HKV * DK], bf16)
                    nc.sync.dma_start(w2k_sb[:], wk2_t)
                    w2v_sb = kv_w2.tile([P, 4, HKV * DK], bf16)
                    nc.sync.dma_start(w2v_sb[:], wv2_t)

                    midk = kv_mid.tile([P, 4, ROWS_PER_CORE], bf16)
                    midv = kv_mid.tile([P, 4, ROWS_PER_CORE], bf16)
                    # piece-major accumulation for the first chunk so the
                    # first matmul only needs DMA piece 0
                    ps_ks = [
                        kv_ps.tile([P, 512], f32, tag="ps1", name=f"ps_k_{rc}")
                        for rc in range(4)
                    ]
                    for dq in range(4):
                        for rc in range(4):
                            for dc in range(4 * dq, 4 * dq + 4):
                                nc.tensor.matmul(
                                    ps_ks[rc][:],
                                    w1k_sb[:, dc, rc * P : (rc + 1) * P],
                                    xtq_ts[dc // 4][:, dc % 4, 0:512],
                                    start=(dc == 0),
                                    stop=(dc == 15),
                                )
                    for rc in range(4):
                        nc.any.tensor_copy(midk[:, rc, 0:512], ps_ks[rc][:])
                    for which in range(3):  # V cn0, K cn1, V cn1
                        w1_sb = w1v_sb if which != 1 else w1k_sb
                        mid = midv if which != 1 else midk
                        cn = 0 if which == 0 else 1
                        for rc in range(4):
                            ps_1 = kv_ps.tile([P, 512], f32, tag="ps1")
                            for dc in range(16):
                                nc.tensor.matmul(
                                    ps_1[:],
                                    w1_sb[:, dc, rc * P : (rc + 1) * P],
                                    xtq_ts[dc // 4][:, dc % 4, cn * 512 : cn * 512 + 512],
                                    start=(dc == 0),
                                    stop=(dc == 15),
                                )
                            nc.any.tensor_copy(
                                mid[:, rc, cn * 512 : cn * 512 + 512], ps_1[:]
                            )
                    # second-stage projections -> collective shard (DRAM)
                    for cn in range(2):
                        for hc in range(HKV):
                            ps_2 = kv_ps.tile([P, 512], f32, tag="ps2")
                            for rc in range(4):
                                nc.tensor.matmul(
                                    ps_2[:],
                                    w2k_sb[:, rc, hc * P : (hc + 1) * P],
                                    midk[:, rc, cn * 512 : cn * 512 + 512],
                                    start=(rc == 0),
                                    stop=(rc == 3),
                                )
                            kb = kv_bounce.tile([P, 512], bf16, tag="kb")
                            nc.any.tensor_copy(kb[:], ps_2[:])
                            nc.sync.dma_start(
                                cc_in[:, hc * SHARD + cn * 512 : hc * SHARD + cn * 512 + 512],
                                kb[:],
                            )
                        for i in range(4):
                            ps_2 = kv_ps.tile([P, 512], f32, tag="ps2")
                            for rc in range(4):
                                nc.tensor.matmul(
                                    ps_2[:],
                                    midv[:, rc, cn * 512 + i * P : cn * 512 + (i + 1) * P],
                                    w2v_sb[:, rc],
                                    start=(rc == 0),
                                    stop=(rc == 3),
                                )
                            vb = kv_bounce.tile([P, 512], bf16, tag="vb")
                            nc.any.tensor_copy(vb[:], ps_2[:])
                            nc.sync.dma_start(
                                cc_in[
                                    :,
                                    CC_K + (cn * 4 + i) * 512 : CC_K + (cn * 4 + i + 1) * 512,
                                ],
                                vb[:],
                            )

                # ------- AllGather kicks off; Q projection hides it -------
                nc.gpsimd.collective_compute(
                    "AllGather",
                    mybir.AluOpType.bypass,
                    replica_groups=[[0, 1], [2, 3], [4, 5], [6, 7]],
                    ins=[cc_in[:].opt()],
                    outs=[cc_out[:].opt()],
                )

                # ------- Phase A2: Q projection (resident) ----------------
                with (
                    tc.tile_pool(name="q_mid", bufs=1) as q_mid,
                    tc.tile_pool(name="q_ps", bufs=4, space="PSUM") as q_ps,
                ):
                    q1t = q_mid.tile([P, 4, ROWS_PER_CORE], bf16)
                    for cn in range(2):
                        for rc in range(4):
                            ps_q = q_ps.tile([P, 512], f32, tag="psq1")
                            for dc in range(16):
                                nc.tensor.matmul(
                                    ps_q[:],
                                    wq1_ts[dc // 4][:, dc % 4, rc * P : (rc + 1) * P],
                                    xtq_ts[dc // 4][:, dc % 4, cn * 512 : cn * 512 + 512],
                                    start=(dc == 0),
                                    stop=(dc == 15),
                                )
                            nc.any.tensor_copy(
                                q1t[:, rc, cn * 512 : cn * 512 + 512], ps_q[:]
                            )
                    for h in range(H):
                        for cn in range(2):
                            ps_qT = q_ps.tile([P, 512], f32, tag="psq2")
                            for rc in range(4):
                                nc.tensor.matmul(
                                    ps_qT[:],
                                    wq2_sb[:, rc, h * P : (h + 1) * P],
                                    q1t[:, rc, cn * 512 : cn * 512 + 512],
                                    start=(rc == 0),
                                    stop=(rc == 3),
                                )
                            nc.any.tensor_copy(
                                qT_all[:, h, cn * 512 : cn * 512 + 512], ps_qT[:]
                            )

            # ------- load gathered K/V shards into SBUF -------------------
            with tc.tile_pool(name="attnkeep", bufs=1) as attnkeep:
                attn_all = attnkeep.tile([P, 8, HKV, 512], bf16)
                # loadbacks gated only by the AllGather; first-needed first
                for r in range(2):
                    nc.sync.dma_start(
                        kT_sb[:, 0, r * SHARD : (r + 1) * SHARD],
                        cc_out[r, :, 0:SHARD],
                    )
                for r in range(2):
                    nc.sync.dma_start(
                        v_sb[:, r * 8 : (r + 1) * 8, :],
                        cc_out[r, :, CC_K:CC_N].rearrange(
                            "p (b c) -> p b c", b=8
                        ),
                    )
                for kvh in range(1, HKV):
                    for r in range(2):
                        nc.sync.dma_start(
                            kT_sb[:, kvh, r * SHARD : (r + 1) * SHARD],
                            cc_out[r, :, kvh * SHARD : (kvh + 1) * SHARD],
                        )

                # --------- Phase 3: attention + Phase 4: Wo ---------------
                with tc.tile_pool(name="wo_w", bufs=2) as wo_w:
                    wo_tiles = {}

                    def fetch_wo(oc):
                        t = wo_w.tile([P, 16, 512], bf16, tag="woc")
                        nc.sync.dma_start(t[:], wo_t[:, :, oc * 512 : (oc + 1) * 512])
                        wo_tiles[oc] = t

                    with (
                        tc.tile_pool(name="at_e", bufs=3) as at_e,
                        tc.tile_pool(name="at_rec", bufs=2) as at_rec,
                        tc.tile_pool(name="at_ps", bufs=2, space="PSUM") as at_ps,
                        tc.tile_pool(name="at_ps_acc", bufs=2, space="PSUM") as at_ps_acc,
                        tc.tile_pool(name="at_ps_sum", bufs=2, space="PSUM") as at_ps_sum,
                    ):
                        for s in range(8):
                            if s == 6:
                                fetch_wo(0)
                            if s == 7:
                                fetch_wo(1)
                            ng = NG[s]
                            for kvh in range(HKV):
                                h0 = 4 * kvh
                                ps_at = at_ps_acc.tile([P, 512], f32, tag="at")
                                ps_sum = at_ps_sum.tile([P, 512], f32, tag="sum")

                                def emit_qk(g, sc_tiles):
                                    masked = g == ng - 1
                                    ps_sc = at_ps.tile([P, 1024], f32, tag="sc")
                                    for j in range(2):
                                        # bank j reads rank j's shard,
                                        # storage position 7-g (shards are
                                        # schedule-ordered = tile-descending)
                                        pos = 7 - g
                                        if masked:
                                            nc.tensor.matmul(
                                                ps_sc[:, j * 512 : (j + 1) * 512],
                                                ident_sb[:],
                                                mask_sb[:, s, j],
                                                start=True,
                                                stop=False,
                                            )
                                        nc.tensor.matmul(
                                            ps_sc[:, j * 512 : (j + 1) * 512],
                                            kT_sb[
                                                :,
                                                kvh,
                                                j * SHARD
                                                + pos * KB : j * SHARD
                                                + (pos + 1) * KB,
                                            ],
                                            qT_all[
                                                :,
                                                h0 : h0 + 4,
                                                s * TILE_R : (s + 1) * TILE_R,
                                            ],
                                            start=not masked,
                                            stop=True,
                                        )
                                    sc_tiles[g] = ps_sc

                                sc_tiles = {}
                                emit_qk(0, sc_tiles)
                                for g in range(ng):
                                    if g + 1 < ng:
                                        emit_qk(g + 1, sc_tiles)
                                    ps_sc = sc_tiles.pop(g)
                                    e_sb = at_e.tile([P, 1024], bf16, tag="e")
                                    nc.scalar.activation(e_sb[:], ps_sc[:], Exp)
                                    for j in range(2):
                                        pos = 7 - g
                                        nc.tensor.matmul(
                                            ps_at[:],
                                            v_sb[
                                                :,
                                                j * 8 + pos,
                                                kvh * DK : (kvh + 1) * DK,
                                            ],
                                            e_sb[:, j * 512 : (j + 1) * 512],
                                            start=(g == 0 and j == 0),
                                            stop=(g == ng - 1 and j == 1),
                                        )
                                    for j in range(2):
                                        nc.tensor.matmul(
                                            ps_sum[:],
                                            ones_sb[:],
                                            e_sb[:, j * 512 : (j + 1) * 512],
                                            start=(g == 0 and j == 0),
                                            stop=(g == ng - 1 and j == 1),
                                        )
                                rec_sb = at_rec.tile([P, 512], f32, tag="rec")
                                nc.vector.reciprocal_approx_fast(
                                    out=rec_sb[:], in_=ps_sum[:]
                                )
                                nc.any.tensor_tensor(
                                    attn_all[:, s, kvh], ps_at[:], rec_sb[:], Mult
                                )

                    # ---- Phase 4: Wo ----
                    with (
                        tc.tile_pool(name="wo_out", bufs=3) as wo_out,
                        tc.tile_pool(name="wo_ps", bufs=3, space="PSUM") as wo_ps,
                    ):
                        for oc in range(4):
                            if oc not in wo_tiles:
                                fetch_wo(oc)
                            wo_sb = wo_tiles[oc]
                            for rc in range(8):
                                ps_o = wo_ps.tile([P, 512], f32, tag="o")
                                for hc in range(16):
                                    nc.tensor.matmul(
                                        ps_o[:],
                                        attn_all[
                                            :,
                                            rc,
                                            hc // 4,
                                            (hc % 4) * P : (hc % 4 + 1) * P,
                                        ],
                                        wo_sb[:, hc],
                                        start=(hc == 0),
                                        stop=(hc == 15),
                                    )
                                o_sb = wo_out.tile([P, 512], f32, tag="osb")
                                nc.any.tensor_copy(o_sb[:], ps_o[:])
                                nc.sync.dma_start(
                                    out[
                                        rc * P : (rc + 1) * P,
                                        oc * 512 : (oc + 1) * 512,
                                    ],
                                    o_sb[:],
                                )

    nc.finalize()
    return nc


def kernel(x, Wq1, Wq2, Wk1, Wk2, Wv1, Wv2, Wo):
    global LAST_RESULT
    from concourse.bass_utils import run_bass_kernel_spmd

    x = np.asarray(x, dtype=np.float32)
    Wq1 = np.asarray(Wq1, dtype=np.float32)
    Wq2 = np.asarray(Wq2, dtype=np.float32)
    Wk1 = np.asarray(Wk1, dtype=np.float32)
    Wk2 = np.asarray(Wk2, dtype=np.float32)
    Wv1 = np.asarray(Wv1, dtype=np.float32)
    Wv2 = np.asarray(Wv2, dtype=np.float32)
    Wo = np.asarray(Wo, dtype=np.float32)

    if "nc" not in _CACHE:
        _CACHE["nc"] = _build_nc()
    nc = _CACHE["nc"]

    wq1_bf = Wq1.astype(BF16)
    wq2_bf = (Wq2 / np.sqrt(DK)).astype(BF16)
    wk1_bf = Wk1.astype(BF16)
    wk2_bf = Wk2.astype(BF16)
    wv1_bf = Wv1.astype(BF16)
    wv2_bf = Wv2.astype(BF16)
    wo_bf = Wo.astype(BF16)
    masks = {p: _make_mask(p) for p in range(2)}
    rows = {p: _rows_sched(p) for p in range(2)}
    ones_np = np.ones((P, P), BF16)
    ident_np = np.eye(P, dtype=np.float32).astype(BF16)

    xT_bf = {}
    for batch in range(B):
        xT_bf[batch] = np.ascontiguousarray(x[batch].T).astype(BF16)

    in_maps = []
    for core in range(8):
        batch, parity = core // 2, core % 2
        xT = xT_bf[batch]
        in_maps.append(
            {
                "xTq": np.ascontiguousarray(xT[:, rows[parity]]),
                "wq1": wq1_bf,
                "wq2": wq2_bf,
                "wk1": wk1_bf,
                "wk2": wk2_bf,
                "wv1": wv1_bf,
                "wv2": wv2_bf,
                "wo": wo_bf,
                "maskin": masks[parity],
                "ones_in": ones_np,
                "ident_in": ident_np,
            }
        )

    res = run_bass_kernel_spmd(nc, in_maps, core_ids=list(range(8)), trace=TRACE)
    LAST_RESULT = res

    out_full = np.empty((B, S, D), np.float32)
    for core in range(8):
        batch, parity = core // 2, core % 2
        out_full[batch][rows[parity]] = res.results[core]["out"]
    return out_full
